# revision 17
# baseline (speedup 1.0000x reference)
"""Causal self-attention on 8 Trainium2 cores (v2).

Sharding: core c handles batch b = c // 2 and head group g = c % 2
(8 of 16 heads, processed as 4 pairs). Wqkv is split column-wise by
head, Wproj row-wise; the host sums the two partial outputs per batch.

v2 structure (vs v1 baseline at 549us):
- QK matmuls for a head pair run CONCURRENTLY on disjoint PE row
  groups (contraction 64 at partitions 0-63 / 64-127 -> tile_position
  (0,0)/(64,0) auto-derived). 2x QK throughput.
- Attention of pair p is emission-interleaved with the q/k projection
  of pair p+1 so the scalar-engine exp stream hides under tensor work
  and the PE never idles past the HAM window (the v1 kernel ran at
  K=4/8 half-clock for 355us).
- One exp per (ci, jt) over a 3D AP covering both heads' PSUM banks.
- Softmax reciprocal on DVE (reciprocal_approx_fast) instead of the
  Ln/Exp table trick: v1 reloaded ACT tables 33x (42us + stalls).
- pt / V / yT / Wproj in bf16: fast weight loads, half the SBUF.
- Causal trimming at 128-column granularity in QK/exp/PV.

Self-contained: hardcodes B=4, L=2048, D=1024, H=16.
"""

import math

import numpy as np
import ml_dtypes

import concourse.bass as bass  # noqa: F401
import concourse.mybir as mybir
import concourse.tile as tile
from concourse import bacc
from concourse.bass_utils import run_bass_kernel_spmd

B, L, D, H, HD = 4, 2048, 1024, 16, 64
N_CORES = 8
HPC = 8            # heads per core
NP = 4             # head pairs per core
KT = D // 128      # 8 contraction tiles
CIW = 512          # query-chunk width
NMT = L // 128     # 16 token/key tiles
PVLAG = 6          # jt lag between QK/exp and PV

f32 = mybir.dt.float32
f32r = mybir.dt.float32r
bf16 = mybir.dt.bfloat16
fp16 = mybir.dt.float16
EXP = mybir.ActivationFunctionType.Exp

_DBG = None  # optional dict of debug DRAM APs (set by debug builds)


def _emit(nc, tc, xT, wqkv, wproj, tri, out):
    with tc.tile_pool(name="persist", bufs=1) as persist:
        # x^T resident: 8 tiles [128 feat, L tok]
        xt = [
            persist.tile([128, L], f32r, tag=f"x{kt}", name=f"x{kt}")
            for kt in range(KT)
        ]
        tri_sb = persist.tile([128, 128], fp16, tag="tri")
        nc.sync.dma_start(out=tri_sb[:], in_=tri[:, :])
        # v in natural layout [key, head, hd+1]; col 64 = 1.0 so the PV
        # matmul also emits the softmax denominator (psum row 64).
        vones = [
            persist.tile([128, HPC, HD + 1], fp16, tag=f"vo{mt}", name=f"vo{mt}")
            for mt in range(NMT)
        ]
        # unnormalized attention output, transposed [feat(2 heads), tok]
        yT = [
            persist.tile([128, L], f32r, tag=f"yT{p}", name=f"yT{p}")
            for p in range(NP)
        ]


        with (
            tc.tile_pool(name="qk", bufs=2) as qkpool,
            tc.tile_pool(name="pt", bufs=2 + PVLAG) as ptpool,
            tc.tile_pool(name="nrm", bufs=2) as nrmpool,
            tc.tile_pool(name="sp", bufs=2, space="PSUM") as sppool,
            tc.tile_pool(name="yp", bufs=1, space="PSUM") as yppool,
        ):
          with (
            tc.tile_pool(name="wqk", bufs=2) as wqkpool,
            tc.tile_pool(name="p1ps", bufs=2, space="PSUM") as p1ps,
          ):
            # ---- v projection (all 8 heads at once, N=512) ----
            with tc.tile_pool(name="wv", bufs=1) as wvpool:
                # interleave wv/xt loads so the kt=0 matmuls start early
                wv = [
                    wvpool.tile([128, 512], f32r, tag=f"wv{kt}", name=f"wv{kt}")
                    for kt in range(KT)
                ]
                for kt in range(KT):
                    nc.sync.dma_start(
                        out=wv[kt][:],
                        in_=wqkv[kt * 128:(kt + 1) * 128, 0:512].bitcast(f32r),
                    )
                    nc.sync.dma_start(
                        out=xt[kt][:, 0:512],
                        in_=xT[kt * 128:(kt + 1) * 128, 0:512].bitcast(f32r),
                    )
                for mc in range(1, 4):
                    for kt in range(KT):
                        nc.sync.dma_start(
                            out=xt[kt][:, mc * 512:(mc + 1) * 512],
                            in_=xT[
                                kt * 128:(kt + 1) * 128, mc * 512:(mc + 1) * 512
                            ].bitcast(f32r),
                        )
                for mt in range(NMT):
                    ps = p1ps.tile([128, 512], f32, tag="p1", name="p1v")
                    for kt in range(KT):
                        nc.tensor.matmul(
                            ps[:],
                            xt[kt][:, mt * 128:(mt + 1) * 128],
                            wv[kt][:],
                            start=(kt == 0),
                            stop=(kt == KT - 1),
                        )
                    nc.vector.tensor_copy(
                        vones[mt][:, :, 0:HD],
                        ps[:].rearrange("p (h d) -> p h d", d=HD),
                    )
                    nc.gpsimd.memset(vones[mt][:, :, HD], 1.0)

            wqk = {}
            qT = {}
            kT = {}

            def load_wqk(p):
                tiles = []
                for kt in range(KT):
                    t = wqkpool.tile(
                        [128, 256], f32r, tag=f"wq{kt}", name=f"wq{kt}p{p}"
                    )
                    base = 512 + 256 * p
                    nc.sync.dma_start(
                        out=t[:],
                        in_=wqkv[
                            kt * 128:(kt + 1) * 128, base:base + 256
                        ].bitcast(f32r),
                    )
                    tiles.append(t)
                wqk[p] = tiles
                qT[p] = qkpool.tile([128, L], f32r, tag="qT", name=f"qT{p}")
                kT[p] = qkpool.tile([128, L], f32r, tag="kT", name=f"kT{p}")

            def emit_qk_sub(p, sub):
                # sub: 0..3 -> (nt, mc-group). 16 matmuls, kt-contiguous.
                nt, mcg = sub // 2, sub % 2
                dst = qT[p] if nt == 0 else kT[p]
                mcs = (2 * mcg, 2 * mcg + 1)
                pss = [
                    p1ps.tile([128, 512], f32, tag="p1", name=f"p1qk{p}{sub}{i}")
                    for i in range(2)
                ]
                for kt in range(KT):
                    for i, mc in enumerate(mcs):
                        nc.tensor.matmul(
                            pss[i][:],
                            wqk[p][kt][:, nt * 128:(nt + 1) * 128],
                            xt[kt][:, mc * 512:(mc + 1) * 512],
                            start=(kt == 0),
                            stop=(kt == KT - 1),
                        )
                for i, mc in enumerate(mcs):
                    nc.vector.tensor_copy(
                        dst[:, mc * 512:(mc + 1) * 512], pss[i][:]
                    )

            def emit_attn_ci(p, ci):
                njt = 4 * (ci + 1)
                ypA = yppool.tile([65, CIW], f32, tag="ypA", name=f"ypA{p}{ci}")
                ypB = yppool.tile([65, CIW], f32, tag="ypB", name=f"ypB{p}{ci}")
                pts = {}

                def emit_qk_exp(jt):
                    off = 128 * jt - CIW * ci
                    a = max(off, 0)
                    aq = min(a, 256)  # keep f32r moving dim >= 256
                    sp = sppool.tile([128, 2, CIW], f32, tag="sp", name="spt")
                    nc.tensor.matmul(
                        sp[:, 0, aq:CIW],
                        kT[p][0:64, jt * 128:(jt + 1) * 128],
                        qT[p][0:64, ci * CIW + aq:(ci + 1) * CIW],
                        start=True,
                        stop=True,
                    )
                    nc.tensor.matmul(
                        sp[:, 1, aq:CIW],
                        kT[p][64:128, jt * 128:(jt + 1) * 128],
                        qT[p][64:128, ci * CIW + aq:(ci + 1) * CIW],
                        start=True,
                        stop=True,
                    )
                    pt = ptpool.tile([128, 2, CIW], fp16, tag="pt", name="ptt")
                    nc.scalar.activation(
                        pt[:, :, a:CIW],
                        sp[:, :, a:CIW],
                        EXP,
                        scale=float(1.0 / math.sqrt(HD)),
                    )
                    if off >= 0:
                        nc.vector.tensor_mul(
                            pt[:, 0, a:a + 128], pt[:, 0, a:a + 128], tri_sb[:]
                        )
                        nc.vector.tensor_mul(
                            pt[:, 1, a:a + 128], pt[:, 1, a:a + 128], tri_sb[:]
                        )
                    if _DBG is not None and p == 0 and ci == 0 and jt == 0:
                        nc.sync.dma_start(
                            out=_DBG["dbg_pt"][:, :],
                            in_=pt[:].rearrange("p a b -> p (a b)"),
                        )
                    pts[jt] = (pt, a)

                def emit_pv(jt):
                    pt, a = pts.pop(jt)
                    nc.tensor.matmul(
                        ypA[:, a:CIW],
                        vones[jt][:, 2 * p, :],
                        pt[:, 0, a:CIW],
                        start=(jt == 0),
                        stop=(jt == njt - 1),
                    )
                    nc.tensor.matmul(
                        ypB[:, a:CIW],
                        vones[jt][:, 2 * p + 1, :],
                        pt[:, 1, a:CIW],
                        start=(jt == 0),
                        stop=(jt == njt - 1),
                    )

                for jt in range(njt + PVLAG):
                    if jt < njt:
                        emit_qk_exp(jt)
                    if jt - PVLAG >= 0:
                        emit_pv(jt - PVLAG)

                # normalize + drain: 1/rowsum via DVE (row 64 of yp), then
                # yT = yp[0:64] * (1/r) fused into the PSUM->SBUF copy.
                for h, yp in ((0, ypA), (1, ypB)):
                    # reciprocal_approx_fast misreads PSUM input; stage the
                    # rowsum to SBUF partition 0 first.
                    rsb = nrmpool.tile([1, CIW], f32, tag="rsb", name="rsbt")
                    nc.vector.tensor_copy(rsb[:], yp[64:65, :])
                    riv = nrmpool.tile([1, CIW], f32, tag="riv", name="rivt")
                    nc.vector.reciprocal_approx_fast(out=riv[:], in_=rsb[:])
                    if _DBG is not None and p == 0 and ci == 0 and h == 0:
                        nc.sync.dma_start(out=_DBG["dbg_r"][:, :], in_=rsb[:])
                        nc.sync.dma_start(out=_DBG["dbg_riv"][:, :], in_=riv[:])
                    bc = nrmpool.tile([64, CIW], f32, tag="bc", name="bct")
                    nc.gpsimd.partition_broadcast(bc[:], riv[:], channels=64)
                    nc.vector.tensor_mul(
                        yT[p][64 * h:64 * h + 64, ci * CIW:(ci + 1) * CIW],
                        yp[0:64, :],
                        bc[:],
                    )

            # ---- pipelined pairs 0..2 (attention p || q/k proj p+1) ----
            load_wqk(0)
            for sub in range(4):
                emit_qk_sub(0, sub)
            if _DBG is not None:
                nc.sync.dma_start(
                    out=_DBG["dbg_q"][:, :], in_=qT[0][:].bitcast(f32)
                )
                nc.sync.dma_start(
                    out=_DBG["dbg_k"][:, :], in_=kT[0][:].bitcast(f32)
                )
            for p in range(NP - 1):
                load_wqk(p + 1)
                for ci in range(4):
                    emit_attn_ci(p, ci)
                    emit_qk_sub(p + 1, ci)
            if _DBG is not None:
                nc.sync.dma_start(out=_DBG["dbg_yT"][:, :], in_=yT[0][:].bitcast(f32))
                nc.sync.dma_start(
                    out=_DBG["dbg_v"][:, :],
                    in_=vones[0][:].rearrange("p h d -> p (h d)"),
                )

          # ---- pair 3 attention || output projection ----
          # (wqk/p1ps closed: their SBUF/PSUM space feeds the P3 pools)
          with (
            tc.tile_pool(name="wp", bufs=1) as wppool,
            tc.tile_pool(name="op", bufs=2) as opool,
            tc.tile_pool(name="p3ps", bufs=2, space="PSUM") as p3ps,
          ):
            wp = []
            for p in range(NP):
                t = wppool.tile([128, D], f32r, tag=f"wp{p}", name=f"wp{p}")
                nc.sync.dma_start(
                    out=t[:],
                    in_=wproj[p * 128:(p + 1) * 128, :].bitcast(f32r),
                )
                wp.append(t)

            def emit_p3_block(ci):
                # output token tiles whose yT[3] chunk just completed
                for it in range(4 * ci, 4 * ci + 4):
                    o_t = opool.tile([128, D], f32, tag="o", name=f"o{it}")
                    for ncol in range(2):
                        ps = p3ps.tile([128, 512], f32, tag="p3", name="p3t")
                        for p in range(NP):
                            nc.tensor.matmul(
                                ps[:],
                                yT[p][:, it * 128:(it + 1) * 128],
                                wp[p][:, ncol * 512:(ncol + 1) * 512],
                                start=(p == 0),
                                stop=(p == NP - 1),
                            )
                        nc.vector.tensor_copy(
                            o_t[:, ncol * 512:(ncol + 1) * 512], ps[:]
                        )
                    nc.sync.dma_start(
                        out=out[it * 128:(it + 1) * 128, :], in_=o_t[:]
                    )

            cis = [3, 2, 1, 0]
            for i, ci in enumerate(cis):
                emit_attn_ci(NP - 1, ci)
                if i > 0:
                    emit_p3_block(cis[i - 1])
            emit_p3_block(cis[-1])


def build(dbg=False):
    global _DBG
    nc = bacc.Bacc(
        "TRN2", target_bir_lowering=False, debug=False, num_devices=N_CORES
    )
    xT = nc.dram_tensor("xT", [D, L], f32, kind="ExternalInput").ap()
    wqkv = nc.dram_tensor("wqkv", [D, 1536], f32, kind="ExternalInput").ap()
    wproj = nc.dram_tensor("wproj", [512, D], f32, kind="ExternalInput").ap()
    tri = nc.dram_tensor("trimask", [128, 128], fp16, kind="ExternalInput").ap()
    out = nc.dram_tensor("out", [L, D], f32, kind="ExternalOutput").ap()
    if dbg:
        _DBG = {
            "dbg_yT": nc.dram_tensor("dbg_yT", [128, L], f32, kind="ExternalOutput").ap(),
            "dbg_q": nc.dram_tensor("dbg_q", [128, L], f32, kind="ExternalOutput").ap(),
            "dbg_k": nc.dram_tensor("dbg_k", [128, L], f32, kind="ExternalOutput").ap(),
            "dbg_v": nc.dram_tensor("dbg_v", [128, 8 * 65], fp16, kind="ExternalOutput").ap(),
            "dbg_pt": nc.dram_tensor("dbg_pt", [128, 2 * CIW], fp16, kind="ExternalOutput").ap(),
            "dbg_r": nc.dram_tensor("dbg_r", [1, CIW], f32, kind="ExternalOutput").ap(),
            "dbg_riv": nc.dram_tensor("dbg_riv", [1, CIW], f32, kind="ExternalOutput").ap(),
        }
    try:
        with tile.TileContext(nc) as tc:
            _emit(nc, tc, xT, wqkv, wproj, tri, out)
        nc.compile()
    finally:
        _DBG = None
    return nc


def shard_inputs(x, Wqkv, Wproj):
    tri = np.triu(np.ones((128, 128), np.float32)).astype(np.float16)
    in_maps = []
    for c in range(N_CORES):
        b, g = c // 2, c % 2
        qcols = Wqkv[:, 512 * g:512 * g + 512]
        kcols = Wqkv[:, D + 512 * g:D + 512 * g + 512]
        vcols = Wqkv[:, 2 * D + 512 * g:2 * D + 512 * g + 512]
        parts = [vcols]
        for p in range(NP):
            parts.append(qcols[:, 128 * p:128 * p + 128])
            parts.append(kcols[:, 128 * p:128 * p + 128])
        wqkv_c = np.ascontiguousarray(np.concatenate(parts, axis=1))
        wproj_c = np.ascontiguousarray(Wproj[512 * g:512 * g + 512, :])
        in_maps.append(
            {
                "xT": np.ascontiguousarray(x[b].T),
                "wqkv": wqkv_c,
                "wproj": wproj_c,
                "trimask": tri,
            }
        )
    return in_maps


_NC_CACHE = {}


def get_nc():
    if "nc" not in _NC_CACHE:
        _NC_CACHE["nc"] = build()
    return _NC_CACHE["nc"]


def run_sharded(in_maps, **kwargs):
    return run_bass_kernel_spmd(
        get_nc(), in_maps, core_ids=list(range(N_CORES)), **kwargs
    )


def kernel(x, Wqkv, Wproj, attn_mask, key_padding_mask):
    # attn_mask is causal and key_padding_mask is all-False for this
    # problem; both are hardcoded into the device program.
    x = np.asarray(x, dtype=np.float32)
    in_maps = shard_inputs(
        x, np.asarray(Wqkv, dtype=np.float32), np.asarray(Wproj, dtype=np.float32)
    )
    res = run_sharded(in_maps)
    outp = np.empty((B, L, D), dtype=np.float32)
    for b in range(B):
        outp[b] = res.results[2 * b]["out"] + res.results[2 * b + 1]["out"]
    return outp
